# revision 24
# baseline (speedup 1.0000x reference)
"""Differential attention kernel for Trainium2 (8 NeuronCores).

Key algebraic rewrite: out = (attn1 - lam*attn2) @ (x @ Wv) @ Wo
                           = diff_attn @ (x @ (Wv @ Wo)).
The 8192-wide V dimension collapses into W_eff = Wv @ Wo  [512, 512],
cutting total FLOPs ~8x (219 -> 28 GFLOP).

Second reassociation: out = (p @ x) @ W_eff (not p @ (x @ W_eff)), so the
big contraction (p @ x, over the 2048 keys) has no W_eff dependency and
the AllReduce result is only needed for the final tiny [512,512]x[512,512]
projection.

Sharding: 8 cores = 2 batches x 4 q-chunks (512 queries each). Each core
computes the full K-side projections for its batch (small), its q-chunk's
scores/softmax/combine, pxT = (p @ x)^T, and out_chunk = px @ W_eff.
W_eff is computed on-device, sharded 8 ways over the 8192-dim v
contraction (1024 per core), then AllReduced (bf16) across all 8 cores.

The body is split into two stages software-pipelined ACROSS iterations
(stage A: W_eff partial + collective + qkv + scores/combine/transpose;
stage B: pxT + final projection). Emission order A_i, B_{i-1} lets
iteration i's attention fill the PE while iteration i-1's AllReduce
completes, so the collective is off the critical path at steady state.

Numerics: scores Q/K kept in fp32r; x, Wqkv, Wv, Wo, e^s, p, W_eff and U
are bf16 (measured max-rel ~7e-3 vs fp32 reference, tolerance 2e-2). The
1/sum(e1) normalizer is folded into the final PSUM eviction; bv/bo fold
into a host-side constant bias using sum_k(diff_attn[q,:]) == 1 - lam.
"""

import math
from contextlib import ExitStack

import numpy as np
import ml_dtypes

import concourse.bass as bass
from concourse import bacc
import concourse.mybir as mybir
import concourse.tile as tile
from concourse import bass_utils
from concourse.bass import ts, ds
from concourse.masks import make_identity

# Problem shapes (hardcoded per harness contract).
B = 2
S = 2048
D = 512
VDIM = 8192
DM = 512             # output dim
P = 128
QC = 512             # q-chunk per core
VSH = VDIM // 8      # 1024-wide v-slice per core for the W_eff partial
SCALE = 1.0 / math.sqrt(64.0)
LAMBDA_INIT = 0.8
LAYER_INDEX = 0

F32 = mybir.dt.float32
F32R = mybir.dt.float32r
BF16 = mybir.dt.bfloat16
EXP = mybir.ActivationFunctionType.Exp
IDENT = mybir.ActivationFunctionType.Identity
AXX = mybir.AxisListType.X
MUL = mybir.AluOpType.mult
ADD = mybir.AluOpType.add

KD = D // P          # 4 contraction chunks of the model dim
MQ = (2 * D) // P    # 8 m-chunks of the qkv output dim (Q1 Q1 Q2 Q2 K1 K1 K2 K2)
SN = S // 512        # 4 free chunks of S
QB = QC // P         # 4 q-blocks per chunk
NKC = S // P         # 16 k-chunks of 128
WKC = VSH // P       # 8 contraction chunks for the W_eff partial


class Pools:
    pass


def open_pools(tc, ctx):
    po = Pools()
    po.persist = ctx.enter_context(tc.tile_pool(name="persist", bufs=1))
    po.xp = ctx.enter_context(tc.tile_pool(name="xp", bufs=1))
    po.xsp = ctx.enter_context(tc.tile_pool(name="xsp", bufs=2))
    po.pxp = ctx.enter_context(tc.tile_pool(name="pxp", bufs=2))
    po.wvp = ctx.enter_context(tc.tile_pool(name="wvp", bufs=1))
    po.wefp = ctx.enter_context(tc.tile_pool(name="wefp", bufs=2))
    po.e1p = ctx.enter_context(tc.tile_pool(name="e1p", bufs=2))
    po.e2p = ctx.enter_context(tc.tile_pool(name="e2p", bufs=2))
    po.smallp = ctx.enter_context(tc.tile_pool(name="smallp", bufs=3))
    po.r1p = ctx.enter_context(tc.tile_pool(name="r1p", bufs=2 * QB))
    po.pbp = ctx.enter_context(tc.tile_pool(name="pbp", bufs=2))
    po.ptp = ctx.enter_context(tc.tile_pool(name="ptp", bufs=2))
    po.ofp = ctx.enter_context(tc.tile_pool(name="ofp", bufs=1))
    po.wps = ctx.enter_context(tc.tile_pool(name="wps", bufs=4, space="PSUM"))
    po.fps = ctx.enter_context(tc.tile_pool(name="fps", bufs=QB, space="PSUM"))
    return po


def emit_const_setup(tc, po):
    nc = tc.nc
    ident_f32 = po.persist.tile([P, P], F32, name="ident_f32")
    po.ident = po.persist.tile([P, P], BF16, name="ident")
    make_identity(nc, ident_f32)
    nc.vector.tensor_copy(po.ident, ident_f32)


def emit_stage_a(tc, po, i, xT, xr, wq, wvT, wo, lam, bq, web_in, web_out):
    """W_eff partial + AllReduce issue + qkv + scores/combine/transpose."""
    nc = tc.nc
    st = Pools()
    st.i = i
    st.web_out = web_out

    # ---- DMA loads (first-consumption order) ----
    lam_sb = po.smallp.tile([P, 1], F32, tag="lam", name=f"lam_{i}")
    bq_sb = po.smallp.tile([P, MQ], F32, tag="bq", name=f"bq_{i}")
    nc.sync.dma_start(lam_sb, lam)
    nc.sync.dma_start(bq_sb, bq)
    wvT_sb = po.wvp.tile([P, WKC, DM], BF16, tag="wvT", name=f"wvT_{i}")
    wo_sb = po.wvp.tile([P, WKC, DM], BF16, tag="wo", name=f"wo_{i}")
    nc.sync.dma_start(wvT_sb, wvT.rearrange("(c p) m -> p c m", p=P))
    nc.sync.dma_start(wo_sb, wo.rearrange("(c p) m -> p c m", p=P))
    xbf = po.xp.tile([P, KD, S], BF16, tag="xbf", name=f"xbf_{i}")
    wq_sb = po.persist.tile([P, KD, 2 * D], BF16, tag="wq", name=f"wq_{i}")
    wqr = wq.rearrange("(c p) m -> p c m", p=P)
    nc.sync.dma_start(wq_sb[:, :, D:], wqr[:, :, D:])
    for dc in range(KD):
        nc.sync.dma_start(xbf[:, dc, ts(0, 512)], xT[ds(dc * P, P), ts(0, 512)])
    nc.sync.dma_start(wq_sb[:, :, :D], wqr[:, :, :D])
    for dc in range(KD):
        nc.sync.dma_start(xbf[:, dc, 512:], xT[ds(dc * P, P), 512:])
    xsb = po.xsp.tile([P, NKC, D], BF16, tag="xsb", name=f"xsb_{i}")
    nc.sync.dma_start(xsb, xr.rearrange("(c p) m -> p c m", p=P))
    st.xsb = xsb

    # ---- W_eff partial (this core's 1024-wide v-slice) + AllReduce ----
    wef_part = po.wefp.tile([P, KD, DM], BF16, tag="wef_part",
                            name=f"wef_part_{i}")
    for db in range(KD):
        pt = po.wps.tile([P, DM], F32, tag="ps", name=f"wps_{i}_{db}")
        for kc in range(WKC):
            nc.tensor.matmul(
                pt, wvT_sb[:, kc, ts(db, P)], wo_sb[:, kc, :],
                start=(kc == 0), stop=(kc == WKC - 1))
        nc.vector.tensor_copy(wef_part[:, db], pt)
    nc.sync.dma_start(web_in, wef_part.rearrange("p c m -> p (c m)"))
    nc.gpsimd.collective_compute(
        "AllReduce", ADD, replica_groups=[list(range(8))],
        ins=[web_in], outs=[web_out])

    # ---- qkv projections (K side: full S; Q side: this q-chunk) ----
    qkvT_K = po.persist.tile([P, 4, S], F32R, tag="qkvK", name=f"qkvK_{i}")
    qkvT_Q = po.persist.tile([P, 4, QC], F32R, tag="qkvQ", name=f"qkvQ_{i}")
    # dc-outer / sn-inner: each wq stationary is loaded once per 4 matmuls
    # (4 psum banks accumulate the 4 sn-chunks of one m-slice in parallel)
    for m in range(4, MQ):
        pts = [po.wps.tile([P, 512], F32, tag="ps", name=f"qk_{i}_{sn}_{m}")
               for sn in range(SN)]
        for dc in range(KD):
            for sn in range(SN):
                nc.tensor.matmul(
                    pts[sn], wq_sb[:, dc, ts(m, P)], xbf[:, dc, ts(sn, 512)],
                    start=(dc == 0), stop=(dc == KD - 1))
        for sn in range(SN):
            # bias-add eviction on DVE (keeps ACT free for the exp evictions)
            nc.vector.tensor_scalar_add(qkvT_K[:, m - 4, ts(sn, 512)], pts[sn],
                                        bq_sb[:, m : m + 1])
    for m in range(4):
        pt = po.wps.tile([P, 512], F32, tag="ps", name=f"qq_{i}_{m}")
        for dc in range(KD):
            nc.tensor.matmul(
                pt, wq_sb[:, dc, ts(m, P)], xbf[:, dc, 0:QC],
                start=(dc == 0), stop=(dc == KD - 1))
        nc.scalar.activation(qkvT_Q[:, m], pt, IDENT,
                             bias=bq_sb[:, m : m + 1])
    # (host rolled x so this core's q-chunk is the first QC columns; the
    # k-order of scores and U use the same roll, which cancels in p^T.T @ U)

    # ---- scores / softmax / combine / transpose ----
    ptile = po.ptp.tile([P, NKC, QC], BF16, tag="pt", name=f"pt_{i}")
    st.ptile = ptile
    st.r1s = []
    pend = []

    def emit_scores(qb):
        ets = []
        sums = []
        for mi in range(2):
            pool = po.e1p if mi == 0 else po.e2p
            et = pool.tile([P, S], BF16, tag=f"e{mi}", name=f"e{mi}_{i}_{qb}")
            stt = po.smallp.tile([P, SN], F32, tag=f"sum{mi}",
                                 name=f"sum{mi}_{i}_{qb}")
            # dc-outer / kn-inner: the Q-block stationary loads once per 4
            # matmuls instead of once per matmul
            pts = [po.wps.tile([P, 512], F32, tag="ps",
                               name=f"ps_{i}_{qb}_{mi}_{kn}")
                   for kn in range(SN)]
            for dc in range(2):
                for kn in range(SN):
                    nc.tensor.matmul(
                        pts[kn],
                        qkvT_Q[:, 2 * mi + dc, ts(qb, P)],
                        qkvT_K[:, 2 * mi + dc, ts(kn, 512)],
                        start=(dc == 0), stop=(dc == 1))
            for kn in range(SN):
                nc.scalar.activation(
                    et[:, ts(kn, 512)], pts[kn], EXP, scale=SCALE,
                    accum_out=stt[:, kn : kn + 1])
            ets.append(et)
            sums.append(stt)
        s1 = po.smallp.tile([P, 1], F32, tag="s1", name=f"s1_{i}_{qb}")
        nc.vector.reduce_sum(s1, sums[0], axis=AXX)
        r1 = po.r1p.tile([P, 1], F32, tag="r1", name=f"r1_{i}_{qb}")
        nc.vector.reciprocal_approx_fast(r1, s1)
        st.r1s.append(r1)
        s2 = po.smallp.tile([P, 1], F32, tag="s2", name=f"s2_{i}_{qb}")
        nc.vector.reduce_sum(s2, sums[1], axis=AXX)
        r2 = po.smallp.tile([P, 1], F32, tag="r2", name=f"r2_{i}_{qb}")
        nc.vector.reciprocal_approx_fast(r2, s2)
        # lam_sb holds -lam, so r2q = -lam*s1/s2 and the combine is a
        # single fused multiply-add: p = e2*r2q + e1.
        u = po.smallp.tile([P, 1], F32, tag="u", name=f"u_{i}_{qb}")
        nc.vector.tensor_mul(u, s1, lam_sb)
        r2q = po.smallp.tile([P, 1], F32, tag="r2q", name=f"r2q_{i}_{qb}")
        nc.vector.tensor_mul(r2q, u, r2)
        pend.append((qb, ets, r2q))

    def emit_combine():
        qb, ets, r2q = pend.pop(0)
        pb = po.pbp.tile([P, S], BF16, tag="pb", name=f"pb_{i}_{qb}")
        for kn in range(SN):
            ks = ts(kn, 512)
            nc.vector.scalar_tensor_tensor(
                pb[:, ks], ets[1][:, ks], r2q, ets[0][:, ks],
                op0=MUL, op1=ADD)
        # 8 transposes per PSUM bank (2KB bf16) -> half the eviction copies
        for kc8 in range(NKC // 8):
            tp = po.wps.tile([P, 8, P], BF16, tag="ps",
                             name=f"tp_{i}_{qb}_{kc8}")
            for j in range(8):
                kc = kc8 * 8 + j
                nc.tensor.matmul(tp[:, j], pb[:, ts(kc, P)], po.ident,
                                 is_transpose=True)
            nc.vector.tensor_copy(ptile[:, ts(kc8, 8), ts(qb, P)], tp)

    for qb in range(QB):
        emit_scores(qb)
        if qb > 0:
            emit_combine()
    emit_combine()
    return st


def emit_stage_b(tc, po, st, out):
    """pxT = (p @ x)^T, out = (px @ W_eff) scaled by 1/s1."""
    nc = tc.nc
    i = st.i
    wef = po.wefp.tile([P, KD, DM], BF16, tag="wef", name=f"wef_{i}")
    nc.sync.dma_start(wef.rearrange("p c m -> p (c m)"), st.web_out)
    pxT = po.pxp.tile([P, KD, QC], BF16, tag="pxT", name=f"pxT_{i}")
    for db in range(KD):
        pt = po.fps.tile([P, QC], F32, tag="f", name=f"px_{i}_{db}")
        for kc in range(NKC):
            nc.tensor.matmul(
                pt, st.xsb[:, kc, ts(db, P)], st.ptile[:, kc, :],
                start=(kc == 0), stop=(kc == NKC - 1))
        nc.vector.tensor_copy(pxT[:, db], pt)
    ofsb = po.ofp.tile([P, QB, DM], F32, tag="of", name=f"of_{i}")
    for qb in range(QB):
        ft = po.wps.tile([P, DM], F32, tag="ps", name=f"ft_{i}_{qb}")
        for dc in range(KD):
            nc.tensor.matmul(
                ft, pxT[:, dc, ts(qb, P)], wef[:, dc, :],
                start=(dc == 0), stop=(dc == KD - 1))
        nc.scalar.activation(ofsb[:, qb], ft, IDENT, scale=st.r1s[qb])
    nc.sync.dma_start(out.rearrange("(c p) m -> p c m", p=P), ofsb)


def build_module(n_iters=1, phases="full"):
    nc = bacc.Bacc("TRN2", target_bir_lowering=False, debug=False)
    xT = nc.dram_tensor("xT", (D, S), BF16, kind="ExternalInput").ap()
    xr = nc.dram_tensor("xr", (S, D), BF16, kind="ExternalInput").ap()
    wq = nc.dram_tensor("wq", (D, 2 * D), BF16, kind="ExternalInput").ap()
    wvT = nc.dram_tensor("wvT", (VSH, DM), BF16, kind="ExternalInput").ap()
    wo = nc.dram_tensor("wo", (VSH, DM), BF16, kind="ExternalInput").ap()
    lam = nc.dram_tensor("lam", (P, 1), F32, kind="ExternalInput").ap()
    bq = nc.dram_tensor("bq", (P, MQ), F32, kind="ExternalInput").ap()
    out = nc.dram_tensor("out", (QC, DM), F32, kind="ExternalOutput").ap()
    with tile.TileContext(nc) as tc, ExitStack() as ctx:
        po = open_pools(tc, ctx)
        emit_const_setup(tc, po)
        prev = None
        for i in range(n_iters):
            web_in = nc.dram_tensor(f"web_in_{i}", (P, KD * DM), BF16).ap()
            web_out = nc.dram_tensor(f"web_out_{i}", (P, KD * DM), BF16,
                                     addr_space="Shared").ap()
            st = emit_stage_a(tc, po, i, xT, xr, wq, wvT, wo, lam, bq, web_in, web_out)
            if prev is not None:
                emit_stage_b(tc, po, prev, out)
            prev = st
        emit_stage_b(tc, po, prev, out)
    nc.compile()
    return nc


_NC = None


def _get_module():
    global _NC
    if _NC is None:
        _NC = build_module()
    return _NC


def host_prep(**inputs):
    """Host-side input prep: returns (in_maps, lam, host_bias)."""
    x = np.asarray(inputs["x"], np.float32)
    Wqkv = np.asarray(inputs["Wqkv"], np.float32)
    bqkv = np.asarray(inputs["bqkv"], np.float32)
    Wv = np.asarray(inputs["Wv"], np.float32)
    bv = np.asarray(inputs["bv"], np.float32)
    Wo = np.asarray(inputs["Wo"], np.float32)
    bo = np.asarray(inputs["bo"], np.float32)
    lq1 = np.asarray(inputs["lq1"], np.float32)
    lk1 = np.asarray(inputs["lk1"], np.float32)
    lq2 = np.asarray(inputs["lq2"], np.float32)
    lk2 = np.asarray(inputs["lk2"], np.float32)

    lam = float(
        np.exp(np.sum(lq1 * lk1, dtype=np.float32))
        - np.exp(np.sum(lq2 * lk2, dtype=np.float32))
        + (LAMBDA_INIT - 0.6 * math.exp(-0.3 * LAYER_INDEX))
    )
    bq_host = np.ascontiguousarray(bqkv.reshape(MQ, P).T)
    # device gets -lam so the combine is a fused multiply-add
    lam_host = np.full((P, 1), -lam, np.float32)
    bf = ml_dtypes.bfloat16
    wq_host = Wqkv.astype(bf)

    in_maps = []
    for c in range(8):
        b, qc = divmod(c, 4)
        # Roll x's sequence dim so this core's q-chunk occupies the first QC
        # columns of xT. K-side scores and U use the same rolled order, so
        # the roll cancels in the k-contraction of p^T.T @ U.
        xr = np.roll(x[b], -qc * QC, axis=0)
        in_maps.append({
            "xT": np.ascontiguousarray(xr.T).astype(bf),
            "xr": xr.astype(bf),
            "wq": wq_host,
            "wvT": np.ascontiguousarray(Wv[:, c * VSH : (c + 1) * VSH].T).astype(bf),
            "wo": np.ascontiguousarray(Wo[c * VSH : (c + 1) * VSH, :]).astype(bf),
            "lam": lam_host,
            "bq": bq_host,
        })
    # sum_k diff_attn[q, :] == 1 - lam exactly, so bv and bo fold into a
    # constant per-output-column correction.
    host_bias = ((1.0 - lam) * bv) @ Wo + bo
    return in_maps, lam, host_bias.astype(np.float32)


def kernel(**inputs):
    in_maps, _lam, host_bias = host_prep(**inputs)
    nc = _get_module()
    res = bass_utils.run_bass_kernel_spmd(nc, in_maps, core_ids=list(range(8)))
    out = np.empty((B, S, DM), np.float32)
    for c in range(8):
        b, qc = divmod(c, 4)
        out[b, qc * QC : (qc + 1) * QC, :] = res.results[c]["out"]
    out += host_bias
    return out
